# revision 14
# baseline (speedup 1.0000x reference)
"""Trainium2 Bass kernel for a dense transformer block (B=2, T=2048, C=1024,
H=16, HID=4096), distributed over 8 NeuronCores.

Sharding: data-parallel over batch (2 groups of 4 cores) x sequence-parallel
over tokens within each group (512 tokens/core). Each core computes local
K^T/V, one AllGather per group shares K^T (feature-major) + V (token-major),
then each core runs full attention for its 512 queries over all 16 heads,
followed by proj/LN1/MLP/LN2 for its tokens. No other collectives needed;
output slices are disjoint.

Matmuls run in float32r (full-rate, fp32 storage, ~1.5e-4 relerr); the
attention P*V product and lin2 use bf16 operands to relieve SBUF pressure.
"""

import numpy as np

import concourse.bass as bass
import concourse.mybir as mybir
import concourse.tile as tile
from concourse import bacc
from concourse.bass_utils import run_bass_kernel_spmd
from concourse.masks import make_identity

# problem dims (hardcoded per contest rules)
B, T, C, H = 2, 2048, 1024, 16
D = C // H            # 64
HID = 4096
TL = T // 4           # 512 tokens per core
NT = TL // 128        # 4 token tiles
CCH = C // 128        # 8 contraction chunks over C
KC = T // 128         # 16 key chunks
JT = HID // 128       # 32 hidden tiles
EPS = 1e-5
NEG = -1.0e9
SCALE = 1.0 / np.sqrt(D)

N_CORES = 8
CORE_IDS = list(range(N_CORES))
RG = [[0, 1, 2, 3], [4, 5, 6, 7]]

f32 = mybir.dt.float32
f32r = mybir.dt.float32r
bf16 = mybir.dt.bfloat16
AF = mybir.ActivationFunctionType

_CACHE = {}


def _build():
    nc = bacc.Bacc("TRN2", target_bir_lowering=False, debug=False,
                   num_devices=N_CORES)

    def inp(name, shape, dt=f32r):
        return nc.dram_tensor(name, shape, dt, kind="ExternalInput").ap()

    xT = inp("xT", [C, TL])
    x_res = inp("x_res", [TL, C], f32)
    wq = inp("wq", [C, C])            # pre-scaled by 1/sqrt(D)
    wk = inp("wk", [C, C])
    wv = inp("wv", [C, C])
    bq_col = inp("bq_col", [D, H], f32)   # pre-scaled
    bk_col = inp("bk_col", [D, H], f32)
    bv_row = inp("bv_row", [1, C])
    qmask = inp("qmask", [2, TL])     # row0 = m_q, row1 = 1-m_q
    kbias = inp("kbias", [2, T])      # row0 = key bias, row1 = onehot bias
    wp = inp("wp", [C, C])
    pb_row = inp("pb_row", [1, C])
    w1 = inp("w1", [C, HID])
    b1_col = inp("b1_col", [128, JT], f32)
    w2 = inp("w2", [HID, C], bf16)
    b2_row = inp("b2_row", [1, C])
    lnw1 = inp("lnw1", [1, C])
    lnb1 = inp("lnb1", [1, C])
    lnw2 = inp("lnw2", [1, C])
    lnb2 = inp("lnb2", [1, C])

    out = nc.dram_tensor("out", [TL, C], f32, kind="ExternalOutput").ap()

    # AG buffers: rows 0..C-1 = K^T local (feature-major, head-grouped),
    # rows C.. = V local (token-major, 2 rows of 512 per token)
    AGR = C + TL * 2  # rows per rank chunk (1536)
    ag_in = nc.dram_tensor("ag_in", [AGR, TL], f32r)
    ag_out = nc.dram_tensor("ag_out", [4 * AGR, TL], f32r)

    with tile.TileContext(nc) as tc:
        pools = {}

        def popen(name, bufs, space="SBUF"):
            cm = tc.tile_pool(name=name, bufs=bufs, space=space)
            pools[name] = cm
            return cm.__enter__()

        def pclose(*names):
            for name in names:
                pools.pop(name).__exit__(None, None, None)

        constp = popen("constp", 1)
        ytp_pool = popen("ytp_pool", 1)   # yt_all: created ph2, used ph3
        stagep = popen("stagep", 2)

        # ---------------- constants ----------------
        ident = constp.tile([128, 128], f32, tag="ident")
        make_identity(nc, ident[:])
        ones128 = constp.tile([1, 128], f32r, tag="ones128")
        nc.vector.memset(ones128[:].bitcast(f32), 1.0)
        ones64 = constp.tile([1, 64], f32r, tag="ones64")
        nc.vector.memset(ones64[:].bitcast(f32), 1.0)
        eps_col = constp.tile([128, 1], f32, tag="eps")
        nc.vector.memset(eps_col[:], EPS)

        psO = popen("psO", 1, "PSUM")
        ln_bc = {}
        for nm, rowap in (("w1", lnw1), ("b1", lnb1), ("w2", lnw2),
                          ("b2", lnb2)):
            rsb = stagep.tile([1, C], f32r, tag="lnrow")
            nc.sync.dma_start(rsb[:], rowap[:])
            bps = psO.tile([128, C], f32, tag="lnbc_ps")
            for hh in range(2):
                nc.tensor.matmul(
                    bps[:, hh * 512:(hh + 1) * 512], ones128[:],
                    rsb[:, hh * 512:(hh + 1) * 512], start=True, stop=True)
            bsb = constp.tile([128, C], f32, tag=f"ln_{nm}")
            nc.scalar.copy(bsb[:], bps[:])
            ln_bc[nm] = bsb
        pclose("psO")

        bias_rows = {}
        for nm, rowap in (("bv", bv_row), ("pb", pb_row), ("b2", b2_row)):
            rsb = constp.tile([1, C], f32r if nm != "b2" else f32r,
                              tag=f"row_{nm}")
            nc.sync.dma_start(rsb[:], rowap[:] if nm != "b2" else rowap[:])
            bias_rows[nm] = rsb
        bqc = constp.tile([D, H], f32, tag="bqc")
        nc.sync.dma_start(bqc[:], bq_col[:])
        bkc = constp.tile([D, H], f32, tag="bkc")
        nc.sync.dma_start(bkc[:], bk_col[:])
        b1c = constp.tile([128, JT], f32, tag="b1c")
        nc.sync.dma_start(b1c[:], b1_col[:])
        pclose("stagep")

        # ---------------- phase 1: QKV ----------------
        qtp = popen("qtp", 1)          # lives through phase 2
        p1 = popen("p1", 2, "PSUM")
        s1a = popen("s1a", 2)
        s1b = popen("s1b", 1)

        xt_all = s1b.tile([128, CCH * TL], f32r, tag="xt")  # 16KB/part
        for cc in range(CCH):
            nc.sync.dma_start(xt_all[:, cc * TL:(cc + 1) * TL],
                              xT[cc * 128:(cc + 1) * 128, :])

        def wcol_load(pool, w_ap, g, ncols, tag):
            # load w[:, g*ncols:(g+1)*ncols] as [128, CCH*ncols] tile
            t = pool.tile([128, CCH * ncols], f32r, tag=tag)
            src = w_ap[:, g * ncols:(g + 1) * ncols].rearrange(
                "(cc p) f -> p cc f", p=128)
            dst = t[:].rearrange("p (cc f) -> p cc f", cc=CCH)
            nc.sync.dma_start(dst, src)
            return t

        # K^T local per 2-head group -> ag_in rows [g*128, (g+1)*128)
        for g in range(H // 2):
            wkg = wcol_load(s1a, wk, g, 128, "wkg")
            kps = p1.tile([128, TL], f32, tag="kt_ps")
            for cc in range(CCH):
                nc.tensor.matmul(kps[:], wkg[:, cc * 128:(cc + 1) * 128],
                                 xt_all[:, cc * TL:(cc + 1) * TL],
                                 start=(cc == 0), stop=(cc == CCH - 1))
            kloc = s1a.tile([128, TL], f32r, tag="kloc")
            for s in range(2):
                nc.scalar.activation(
                    kloc[s * 64:(s + 1) * 64, :], kps[s * 64:(s + 1) * 64, :],
                    AF.Identity, bias=bkc[:, 2 * g + s:2 * g + s + 1])
            nc.sync.dma_start(ag_in[g * 128:(g + 1) * 128, :], kloc[:])

        # V local token-major -> ag_in rows C + tt*256 + (2j+half)
        agv = ag_in.ap()[C:C + 2 * TL].rearrange("(t two) f -> t two f", two=2)
        for half in range(2):
            wvh = wcol_load(s1a, wv, half, 512, "wvh")  # [128, CCH*512]
            for tt in range(NT):
                vps = p1.tile([128, 512], f32, tag="v_ps")
                for cc in range(CCH):
                    nc.tensor.matmul(
                        vps[:],
                        xt_all[:, cc * TL + tt * 128:cc * TL + (tt + 1) * 128],
                        wvh[:, cc * 512:(cc + 1) * 512],
                        start=(cc == 0), stop=False)
                nc.tensor.matmul(
                    vps[:], ones128[:],
                    bias_rows["bv"][:, half * 512:(half + 1) * 512],
                    start=False, stop=True)
                vloc = s1a.tile([128, 512], f32r, tag="vloc")
                nc.scalar.copy(vloc[:], vps[:])
                nc.sync.dma_start(
                    agv[tt * 128:(tt + 1) * 128, half, :], vloc[:])

        # AllGather (runs on TOPSP; overlaps with Q^T below)
        nc.gpsimd.collective_compute(
            "AllGather", mybir.AluOpType.bypass,
            ins=[ag_in[:]], outs=[ag_out[:]], replica_groups=RG)

        # Q^T local (+mask rows): qt_all[0:64, h*512:...] per head,
        # rows 64:66 = qmask rows
        qt_all = qtp.tile([66, H * TL], f32r, tag="qt")  # 32KB/part
        for g in range(H // 2):
            wqg = wcol_load(s1a, wq, g, 128, "wqg")
            qps = p1.tile([128, TL], f32, tag="qt_ps")
            for cc in range(CCH):
                nc.tensor.matmul(qps[:], wqg[:, cc * 128:(cc + 1) * 128],
                                 xt_all[:, cc * TL:(cc + 1) * TL],
                                 start=(cc == 0), stop=(cc == CCH - 1))
            for s in range(2):
                h = 2 * g + s
                nc.scalar.activation(
                    qt_all[0:64, h * TL:(h + 1) * TL],
                    qps[s * 64:(s + 1) * 64, :],
                    AF.Identity, bias=bqc[:, h:h + 1])
        for h in range(H):
            nc.sync.dma_start(qt_all[64:66, h * TL:(h + 1) * TL], qmask[:])

        pclose("s1b", "s1a", "p1")

        # ---------------- phase 2: attention ----------------
        p2 = popen("p2", 3, "PSUM")
        p2b = popen("p2b", 2, "PSUM")
        s2a = popen("s2a", 1)
        s2b = popen("s2b", 2)
        s2c = popen("s2c", 3)
        s2d = popen("s2d", 2)

        # V_ext: [128, KC x (H x 66)] bf16; col 64 of each head block = 1
        VW = H * 66  # 1056
        v_all = s2a.tile([128, KC * VW], bf16, tag="v_all")  # 33KB/part
        for kc in range(KC):
            r, loc = kc // 4, kc % 4
            src = ag_out.ap()[r * AGR + C + loc * 256:
                              r * AGR + C + (loc + 1) * 256].rearrange(
                "(t two) f -> t (two f)", two=2).rearrange(
                "t (b f) -> t b f", f=D)
            dst = v_all[:, kc * VW:(kc + 1) * VW].rearrange(
                "p (b f) -> p b f", f=66)
            vstage = s2c.tile([128, C], f32r, tag="vstage")
            nc.sync.dma_start(vstage[:], src.rearrange("t b f -> t (b f)"))
            nc.vector.tensor_copy(
                dst[:, :, 0:64],
                vstage[:].rearrange("t (b f) -> t b f", f=D))
        vre = v_all[:].rearrange("p (a f) -> p a f", f=66)
        nc.vector.memset(vre[:, :, 64:65], 1.0)
        nc.vector.memset(vre[:, :, 65:66], 0.0)

        yt_all = ytp_pool.tile([128, CCH * TL], f32r, tag="yt")  # 16KB/part
        for h in range(H):
            g, s = h // 2, h % 2
            kt = s2b.tile([66, T], f32r, tag="kt_ext")  # 8KB/part
            for r in range(4):
                nc.sync.dma_start(
                    kt[0:64, r * TL:(r + 1) * TL],
                    ag_out.ap()[r * AGR + g * 128 + s * 64:
                                r * AGR + g * 128 + (s + 1) * 64, :])
            nc.sync.dma_start(kt[64:66, :], kbias[:])

            ytps = p2b.tile([66, TL], f32, tag="yt_ps")
            for kc in range(KC):
                stp = p2.tile([128, TL], f32, tag="st_ps")
                nc.tensor.matmul(stp[:], kt[:, kc * 128:(kc + 1) * 128],
                                 qt_all[:, h * TL:(h + 1) * TL],
                                 start=True, stop=True)
                pt = s2c.tile([128, TL], bf16, tag="pt")
                nc.scalar.activation(pt[:], stp[:], AF.Exp)
                nc.tensor.matmul(
                    ytps[:], v_all[:, kc * VW + h * 66:kc * VW + (h + 1) * 66],
                    pt[:], start=(kc == 0), stop=(kc == KC - 1))

            # normalize: yt_all[dst] = ytps[0:64] * (1/s) broadcast
            rec = s2d.tile([1, TL], f32, tag="rec")
            nc.vector.reciprocal(rec[:], ytps[64:65, :])
            recr = s2d.tile([1, TL], f32r, tag="recr")
            nc.scalar.copy(recr[:], rec[:])
            bcp = p2.tile([64, TL], f32, tag="bc_ps")
            nc.tensor.matmul(bcp[:], ones64[:], recr[:], start=True, stop=True)
            bcs = s2d.tile([64, TL], f32, tag="bc_sb")
            nc.scalar.copy(bcs[:], bcp[:])
            dst = yt_all[s * 64:(s + 1) * 64, g * TL:(g + 1) * TL]
            nc.vector.tensor_mul(dst, ytps[0:64, :], bcs[:])

        pclose("s2d", "s2c", "s2b", "s2a", "p2b", "p2", "qtp")

        # ---------------- phase 3: proj + LN1 ----------------
        hhp = popen("hhp", 1)          # h_all + hT_all, live through phase 4
        lnsp = popen("lnsp", 2)        # LN scratch, phases 3+4
        statp = popen("statp", 2)
        p3 = popen("p3", 2, "PSUM")
        s3a = popen("s3a", 1)
        s3b = popen("s3b", 2)

        wpt = s3a.tile([128, CCH * C], f32r, tag="wp")  # 32KB/part
        nc.sync.dma_start(
            wpt[:].rearrange("p (cc f) -> p cc f", cc=CCH),
            wp[:].rearrange("(cc p) f -> p cc f", p=128))

        h_all = hhp.tile([128, NT * C], f32, tag="h_all")    # 16KB
        hT_all = hhp.tile([128, CCH * TL], f32r, tag="hT")   # 16KB

        def layer_norm(r1, w_bc, b_bc, out_ap):
            sq = lnsp.tile([128, C], f32, tag="sq")
            s2t = statp.tile([128, 1], f32, tag="s2t")
            nc.scalar.activation(sq[:], r1[:], AF.Square, accum_out=s2t[:])
            s1t = statp.tile([128, 1], f32, tag="s1t")
            nc.vector.reduce_sum(s1t[:], r1[:], axis=mybir.AxisListType.X)
            nmu = statp.tile([128, 1], f32, tag="nmu")
            nc.vector.tensor_scalar_mul(nmu[:], s1t[:], -1.0 / C)
            var = statp.tile([128, 1], f32, tag="var")
            nc.vector.tensor_mul(var[:], nmu[:], nmu[:])
            nc.vector.tensor_scalar_mul(s2t[:], s2t[:], 1.0 / C)
            nc.vector.tensor_sub(var[:], s2t[:], var[:])
            std = statp.tile([128, 1], f32, tag="std")
            nc.scalar.activation(std[:], var[:], AF.Sqrt, bias=eps_col[:])
            rstd = statp.tile([128, 1], f32, tag="rstd")
            nc.vector.reciprocal(rstd[:], std[:])
            nmr = statp.tile([128, 1], f32, tag="nmr")
            nc.vector.tensor_mul(nmr[:], nmu[:], rstd[:])
            nrm = lnsp.tile([128, C], f32, tag="nrm")
            nc.scalar.activation(nrm[:], r1[:], AF.Identity,
                                 bias=nmr[:], scale=rstd[:])
            nc.vector.tensor_mul(nrm[:], nrm[:], w_bc[:])
            nc.vector.tensor_add(out_ap, nrm[:], b_bc[:])

        for tt in range(NT):
            xr = s3b.tile([128, C], f32, tag="xr")
            nc.sync.dma_start(xr[:], x_res[tt * 128:(tt + 1) * 128, :])
            r1 = s3b.tile([128, C], f32, tag="r1")
            for half in range(2):
                zps = p3.tile([128, 512], f32, tag="z_ps")
                for cc in range(CCH):
                    nc.tensor.matmul(
                        zps[:],
                        yt_all[:, cc * TL + tt * 128:cc * TL + (tt + 1) * 128],
                        wpt[:, cc * C + half * 512:cc * C + (half + 1) * 512],
                        start=(cc == 0), stop=False)
                nc.tensor.matmul(
                    zps[:], ones128[:],
                    bias_rows["pb"][:, half * 512:(half + 1) * 512],
                    start=False, stop=True)
                nc.vector.tensor_add(r1[:, half * 512:(half + 1) * 512],
                                     xr[:, half * 512:(half + 1) * 512],
                                     zps[:])
            layer_norm(r1, ln_bc["w1"], ln_bc["b1"],
                       h_all[:, tt * C:(tt + 1) * C])
            for cc in range(CCH):
                trp = p3.tile([128, 128], f32, tag="tr_ps")
                nc.tensor.transpose(
                    trp[:],
                    h_all[:, tt * C + cc * 128:tt * C + (cc + 1) * 128],
                    ident[:])
                nc.scalar.copy(
                    hT_all[:, cc * TL + tt * 128:cc * TL + (tt + 1) * 128],
                    trp[:])

        pclose("s3b", "s3a", "p3")

        # ---------------- phase 4: MLP + LN2 ----------------
        s4a = popen("s4a", 1)
        p4a = popen("p4a", 2, "PSUM")
        s4b = popen("s4b", 2)

        aT_all = s4a.tile([128, JT * TL], bf16, tag="aT")  # 32KB/part
        for jt in range(JT):
            w1g = wcol_load(s4b, w1, jt, 128, "w1g")  # noqa
            aps = p4a.tile([128, TL], f32, tag="a_ps")
            for cc in range(CCH):
                nc.tensor.matmul(aps[:], w1g[:, cc * 128:(cc + 1) * 128],
                                 hT_all[:, cc * TL:(cc + 1) * TL],
                                 start=(cc == 0), stop=(cc == CCH - 1))
            nc.scalar.activation(aT_all[:, jt * TL:(jt + 1) * TL], aps[:],
                                 AF.Gelu, bias=b1c[:, jt:jt + 1])
        pclose("s4b", "p4a")

        p4b = popen("p4b", 1, "PSUM")
        s4c = popen("s4c", 3)
        s4d = popen("s4d", 2)

        b2_bf = s4d.tile([1, C], bf16, tag="b2bf")
        nc.scalar.copy(b2_bf[:], bias_rows["b2"][:])
        ones128_bf = s4d.tile([1, 128], bf16, tag="ones128bf")
        nc.vector.memset(ones128_bf[:], 1.0)

        fps = []
        for tt in range(NT):
            fp_tile = p4b.tile([128, C], f32, tag=f"f_ps{tt}")
            fps.append(fp_tile)
        for jc in range(JT):
            w2t = s4c.tile([128, C], bf16, tag="w2t")
            nc.sync.dma_start(w2t[:], w2[jc * 128:(jc + 1) * 128, :])
            for tt in range(NT):
                for half in range(2):
                    nc.tensor.matmul(
                        fps[tt][:, half * 512:(half + 1) * 512],
                        aT_all[:, jc * TL + tt * 128:jc * TL + (tt + 1) * 128],
                        w2t[:, half * 512:(half + 1) * 512],
                        start=(jc == 0), stop=False)
        for tt in range(NT):
            for half in range(2):
                nc.tensor.matmul(
                    fps[tt][:, half * 512:(half + 1) * 512], ones128_bf[:],
                    b2_bf[:, half * 512:(half + 1) * 512],
                    start=False, stop=True)

        for tt in range(NT):
            r2 = s4d.tile([128, C], f32, tag="r2")
            nc.vector.tensor_add(r2[:], h_all[:, tt * C:(tt + 1) * C],
                                 fps[tt][:])
            osb = s4d.tile([128, C], f32, tag="osb")
            layer_norm(r2, ln_bc["w2"], ln_bc["b2"], osb[:])
            nc.sync.dma_start(out[tt * 128:(tt + 1) * 128, :], osb[:])

        pclose("s4d", "s4c", "s4a", "statp", "lnsp", "hhp",
               "ytp_pool", "constp", "p4b")

    nc.compile()
    return nc


def _prep_inputs(x, mask, attn_w, attn_b, proj_w, proj_b, ln1_w, ln1_b,
                 lin1_w, lin1_b, lin2_w, lin2_b, ln2_w, ln2_b):
    import ml_dtypes
    f = np.float32
    x = np.asarray(x, f)
    mask = np.asarray(mask)
    attn_w = np.asarray(attn_w, f)
    attn_b = np.asarray(attn_b, f)
    wq_s = np.ascontiguousarray(attn_w[:, :C] * SCALE)
    wk = np.ascontiguousarray(attn_w[:, C:2 * C])
    wv = np.ascontiguousarray(attn_w[:, 2 * C:])
    bq_col = np.ascontiguousarray((attn_b[:C] * SCALE).reshape(H, D).T)
    bk_col = np.ascontiguousarray(attn_b[C:2 * C].reshape(H, D).T)
    bv_row = np.ascontiguousarray(attn_b[2 * C:].reshape(1, C))
    pb_row = np.asarray(proj_b, f).reshape(1, C)
    b1_col = np.ascontiguousarray(np.asarray(lin1_b, f).reshape(JT, 128).T)
    b2_row = np.asarray(lin2_b, f).reshape(1, C)

    common = {
        "wq": wq_s, "wk": wk, "wv": wv, "bq_col": bq_col, "bk_col": bk_col,
        "bv_row": bv_row, "wp": np.asarray(proj_w, f), "pb_row": pb_row,
        "w1": np.asarray(lin1_w, f), "b1_col": b1_col,
        "w2": np.asarray(lin2_w, f).astype(ml_dtypes.bfloat16),
        "b2_row": b2_row,
        "lnw1": np.asarray(ln1_w, f).reshape(1, C),
        "lnb1": np.asarray(ln1_b, f).reshape(1, C),
        "lnw2": np.asarray(ln2_w, f).reshape(1, C),
        "lnb2": np.asarray(ln2_b, f).reshape(1, C),
    }

    in_maps = []
    for c in range(N_CORES):
        b, s = c // 4, c % 4
        tok = slice(s * TL, (s + 1) * TL)
        mb = np.asarray(mask[b]).astype(bool)
        km = mb.copy()
        km[0] = True
        kb = np.zeros((2, T), f)
        kb[0] = np.where(km, 0.0, NEG)
        kb[1] = NEG
        kb[1, 0] = 0.0
        mq = mb[tok].astype(f)
        qm = np.stack([mq, 1.0 - mq]).astype(f)
        m = dict(common)
        m["xT"] = np.ascontiguousarray(x[b, tok, :].T)
        m["x_res"] = np.ascontiguousarray(x[b, tok, :])
        m["qmask"] = qm
        m["kbias"] = kb
        in_maps.append(m)
    return in_maps


def _get_nc():
    if "nc" not in _CACHE:
        _CACHE["nc"] = _build()
    return _CACHE["nc"]


def kernel(**inputs):
    nc = _get_nc()
    in_maps = _prep_inputs(**inputs)
    res = run_bass_kernel_spmd(nc, in_maps, CORE_IDS)
    out = np.empty((B, T, C), np.float32)
    for c in range(N_CORES):
        b, s = c // 4, c % 4
        out[b, s * TL:(s + 1) * TL, :] = res.results[c]["out"]
    return out
